# revision 20
# baseline (speedup 1.0000x reference)
"""Trainium2 Bass kernel for nn_Block_9981503996445 (dense_transformer).

Block: GroupNorm(1,256) -> 3x3 conv 256->384 -> split q/k/v/u ->
depthwise stride-2 downsample of k,v -> attention softmax(q^T k / 8) over
M=1024 -> a = v @ attn^T -> concat(a,u) -> relu -> 1x1 conv + residual ->
FFN (1x1 -> relu -> 1x1) + residual.

Sharding: data-parallel over batch, B=16 -> 2 samples per core on 8 cores.

All heavy matmuls run in fp8(e4m3), using DoubleRow perf mode (K=256 per
pass) wherever the contraction allows: the 3x3 conv (K=2x128 ct groups per
tap), attention a/denominator (K=2x128 m-tiles), wout-u (K=2x64), ffn1
(K=2x128), ffn2 (2x DR over 4 h-tiles).  fp8 scale management:
 - activations carry power-of-2 scales kx/kq/kk/kv/ke/ka/ku/k1/k2, folded
   into PSUM-drain scale/bias vectors ([P,1] per-partition operands of
   ACT/DVE) and into the depthwise downsample multipliers.
 - weights carry per-output-channel power-of-2 scales (lambda), folded out
   at each drain; softmax is scale-invariant so est keeps its ke factor.
 - b_in(k), b_k dropped (per-n logit shifts are softmax-invariant);
   b_in(v), b_v folded into the post-attention relu bias (attn rows sum
   to 1); b_out folded into b1/b2; b2+b_out added via the final ACT drain.
Channel layout: T0=[q;u0], T1=[k;u1], T2=v so q/kf sit at base partition
0 (matmul base match) and u0/u1 relu-drain into one [128,2,N] DR group
tile at partitions 64:128.  GroupNorm rstd uses exp(-0.5*ln(var+eps)) to
stay inside the natural_log_exp activation table (no table reloads).
"""

import sys

for _p in ("/opt/trn_rl_repo",):
    if _p not in sys.path:
        sys.path.insert(0, _p)

from contextlib import ExitStack

import numpy as np
import ml_dtypes

import concourse.bacc as bacc
import concourse.mybir as mybir
import concourse.tile as tile
from concourse.bass_utils import run_bass_kernel_spmd
from concourse.dve_ops import AFFINE_THEN_ADD

F32 = mybir.dt.float32
F32R = mybir.dt.float32r
F8 = mybir.dt.float8e4
E4NP = ml_dtypes.float8_e4m3
DRM = mybir.MatmulPerfMode.DoubleRow
AF = mybir.ActivationFunctionType
OP = mybir.AluOpType

P = 128
DIM, QK, PD, HID = 256, 64, 128, 512
H = W = 64
N = H * W            # 4096
M = 1024             # 32*32 after stride-2 downsample
WP = W + 2           # 66 padded row stride
NPAD = WP * (H + 2)  # 4356
SCALE = QK ** -0.5
EPS = 1e-5
NCORES = 8
SPC = 2              # samples per core
NBLK = 8
BW = N // NBLK       # 512
RB = BW // W         # 8 rows per block

# fp8 activation scales (powers of 2)
KX = 8.0    # xn
KQ = 16.0   # q8
KK0 = 8.0   # k8 (full res)
KK = 16.0   # kf8
KV0 = 8.0   # v8 (full res)
KV = 16.0   # vf8
KE = 32.0   # est8 (exp output)
KA = 16.0   # ra8
KU = 16.0   # ru8
K1 = 16.0   # x18
K2 = 16.0   # h8

_CACHE = {}


def _lam(w, kap, axis_out=0):
    """per-output-channel power-of-2 scale so max|w*lam/kap| ~ [112,224]."""
    ax = tuple(i for i in range(w.ndim) if i != axis_out)
    mx = np.max(np.abs(w) / kap, axis=ax)
    return 2.0 ** np.floor(np.log2(224.0 / np.maximum(mx, 1e-30)))


def _q8(w):
    return np.asarray(w, E4NP)


def _prep_weights(w_in, s_in, b_in, w_k, s_k, w_v, s_v, b_v,
                  w_out, s_out, b_out, w1, s1, b1, w2, s2, b2, gn_w, gn_b):
    f32 = np.float32
    wf = (w_in * s_in[:, None, None, None]).astype(f32)  # [384,256,3,3]

    # channel map: tile t partition o -> reference output channel
    refch = np.empty((3, P), np.int64)
    refch[0, :64] = np.arange(0, 64)        # q
    refch[0, 64:] = np.arange(256, 320)     # u0
    refch[1, :64] = np.arange(64, 128)      # k
    refch[1, 64:] = np.arange(320, 384)     # u1
    refch[2, :] = np.arange(128, 256)       # v

    # per-(tile,partition) weight scale lambda; conv input is xn*KX
    lam_c = np.empty((3, P), f32)
    for t in range(3):
        lam_c[t] = _lam(wf[refch[t]].reshape(P, -1), 1.0)
    WIN8 = np.empty((9, 3, P, 2, P), E4NP)  # [tap, t, c, g(ct), o]
    for ky in range(3):
        for kx in range(3):
            tap = ky * 3 + kx
            for t in range(3):
                # lhsT[c, g, o] = wf[refch(t,o), g*128+c, ky, kx]*lam
                wt = wf[refch[t], :, ky, kx] * lam_c[t][:, None]  # [o, 384.. c]
                wt = wt.T.reshape(2, P, P).transpose(1, 0, 2)     # [c, g, o]
                WIN8[tap, t] = _q8(wt)

    # conv drain scale/bias [3, P]
    outk = np.empty((3, P), f32)
    outk[0, :64] = KQ
    outk[0, 64:] = KU
    outk[1, :64] = KK0
    outk[1, 64:] = KU
    outk[2, :] = KV0
    CSC = (outk / (KX * lam_c)).astype(f32).reshape(3, P, 1)
    CBI = np.zeros((3, P), f32)
    CBI[0, :64] = b_in[0:64] * KQ            # q keeps its bias
    CBI[0, 64:] = b_in[256:320] * KU         # u0
    CBI[1, 64:] = b_in[320:384] * KU         # u1
    # k: bias dropped (per-n logit shift); v: bias folded into ra bias
    CBI = CBI.reshape(3, P, 1)

    # depthwise downsample multipliers [P, 4] (j = dy*2+dx)
    WKD = np.zeros((P, 4), f32)
    WKD[0:64] = (w_k[:, 0] * s_k[:, None, None]).reshape(QK, 4) * (KK / KK0)
    WVD = (w_v[:, 0] * s_v[:, None, None]).reshape(PD, 4) * (KV / KV0)

    # ra bias: kv-const(b_in_v) + b_v, times KA
    dwv = (w_v[:, 0] * s_v[:, None, None]).sum(axis=(1, 2))  # [PD]
    RAB = ((dwv * b_in[128:256] + b_v) * KA).astype(f32).reshape(P, 1)

    # wout
    wo = (w_out[:, :, 0, 0] * s_out[:, None]).astype(f32)  # [256,256]
    wos = np.concatenate([wo[:, 0:PD] / KA, wo[:, PD:] / KU], axis=1)
    lam_o = _lam(wos, 1.0)
    WOA8 = np.empty((2, P, P), E4NP)
    WOU8 = np.zeros((2, P, 2, P), E4NP)
    for ot in range(2):
        rows = slice(ot * P, (ot + 1) * P)
        WOA8[ot] = _q8((wo[rows, 0:PD].T / KA) * lam_o[rows][None, :])
        for g in range(2):
            # u group g channels: cat ch 128+g*64 .. 128+(g+1)*64 live at
            # lhsT partitions 64:128
            wu = wo[rows, PD + g * 64:PD + (g + 1) * 64]  # [128 o, 64 c]
            WOU8[ot, 64:, g, :] = _q8((wu.T / KU) * lam_o[rows][None, :])
    INVLO = (1.0 / lam_o).astype(f32).reshape(2, P, 1)

    # ffn1; b_out folded into b1
    w1f = (w1[:, :, 0, 0] * s1[:, None]).astype(f32)  # [512,256]
    b1p = (b1 + w1f @ b_out).astype(f32)
    lam1 = _lam(w1f / K1, 1.0)
    W18 = np.empty((4, P, 2, P), E4NP)
    for ot in range(4):
        rows = slice(ot * P, (ot + 1) * P)
        wt = (w1f[rows] / K1) * lam1[rows][:, None]   # [o, 256]
        W18[ot] = _q8(wt.T.reshape(2, P, P).transpose(1, 0, 2))
    HSC = (K2 / lam1).astype(f32).reshape(4, P, 1)
    HBI = (b1p * K2).astype(f32).reshape(4, P, 1)

    # ffn2; out drain adds b2 + b_out
    w2f = (w2[:, :, 0, 0] * s2[:, None]).astype(f32)  # [256,512]
    lam2 = _lam(w2f / K2, 1.0)
    W28 = np.empty((2, 2, P, 2, P), E4NP)  # [ot, j, c, g, o]
    for ot in range(2):
        rows = slice(ot * P, (ot + 1) * P)
        for j in range(2):
            wt = (w2f[rows, j * 2 * P:(j + 1) * 2 * P] / K2) \
                * lam2[rows][:, None]
            W28[ot, j] = _q8(wt.T.reshape(2, P, P).transpose(1, 0, 2))
    FSC = (1.0 / lam2).astype(f32).reshape(2, P, 1)
    FBI = (b2 + b_out).astype(f32).reshape(2, P, 1)

    GNW = (gn_w * KX).astype(f32).reshape(2, P, 1)
    GNB = (gn_b * KX).astype(f32).reshape(2, P, 1)

    ONES8 = np.ones((P, 2, 32), E4NP)
    IDENT8 = np.asarray(np.eye(P, dtype=f32), E4NP)

    return dict(
        win8=WIN8.reshape(27, P, 2, P), csc=CSC, cbi=CBI,
        wkd=WKD, wvd=WVD, rab=RAB,
        woa8=WOA8, wou8=WOU8, invlo=INVLO,
        w18=W18, hsc=HSC, hbi=HBI,
        w28=W28.reshape(4, P, 2, P), fsc=FSC, fbi=FBI,
        gnw=GNW, gnb=GNB, ones8=ONES8, ident8=IDENT8,
    )


def _build(n_cores):
    nc = bacc.Bacc("TRN2", target_bir_lowering=False, debug=False,
                   num_devices=n_cores)

    x_d = nc.dram_tensor("x", [SPC, DIM, N], F32, kind="ExternalInput").ap()
    win8_d = nc.dram_tensor("win8", [27, P, 2, P], F8, kind="ExternalInput").ap()
    csc_d = nc.dram_tensor("csc", [3, P, 1], F32, kind="ExternalInput").ap()
    cbi_d = nc.dram_tensor("cbi", [3, P, 1], F32, kind="ExternalInput").ap()
    wkd_d = nc.dram_tensor("wkd", [P, 4], F32, kind="ExternalInput").ap()
    wvd_d = nc.dram_tensor("wvd", [P, 4], F32, kind="ExternalInput").ap()
    rab_d = nc.dram_tensor("rab", [P, 1], F32, kind="ExternalInput").ap()
    woa8_d = nc.dram_tensor("woa8", [2, P, P], F8, kind="ExternalInput").ap()
    wou8_d = nc.dram_tensor("wou8", [2, P, 2, P], F8, kind="ExternalInput").ap()
    invlo_d = nc.dram_tensor("invlo", [2, P, 1], F32, kind="ExternalInput").ap()
    w18_d = nc.dram_tensor("w18", [4, P, 2, P], F8, kind="ExternalInput").ap()
    hsc_d = nc.dram_tensor("hsc", [4, P, 1], F32, kind="ExternalInput").ap()
    hbi_d = nc.dram_tensor("hbi", [4, P, 1], F32, kind="ExternalInput").ap()
    w28_d = nc.dram_tensor("w28", [4, P, 2, P], F8, kind="ExternalInput").ap()
    fsc_d = nc.dram_tensor("fsc", [2, P, 1], F32, kind="ExternalInput").ap()
    fbi_d = nc.dram_tensor("fbi", [2, P, 1], F32, kind="ExternalInput").ap()
    gnw_d = nc.dram_tensor("gnw", [2, P, 1], F32, kind="ExternalInput").ap()
    gnb_d = nc.dram_tensor("gnb", [2, P, 1], F32, kind="ExternalInput").ap()
    ones8_d = nc.dram_tensor("ones8", [P, 2, 32], F8, kind="ExternalInput").ap()
    ident8_d = nc.dram_tensor("ident8", [P, P], F8, kind="ExternalInput").ap()
    out_d = nc.dram_tensor("out", [SPC, DIM, N], F32, kind="ExternalOutput").ap()

    xv_d = x_d.rearrange("s (ct p) n -> s ct p n", p=P)
    ov_d = out_d.rearrange("s (ct p) n -> s ct p n", p=P)

    with tile.TileContext(nc) as tc, ExitStack() as ctx, \
            nc.allow_low_precision(reason="fp8 matmuls"):
        consts = ctx.enter_context(tc.tile_pool(name="consts", bufs=1))
        wp = ctx.enter_context(tc.tile_pool(name="wp", bufs=1))
        big = ctx.enter_context(tc.tile_pool(name="big", bufs=1))
        est_p = ctx.enter_context(tc.tile_pool(name="est", bufs=2))
        small = ctx.enter_context(tc.tile_pool(name="small", bufs=2))
        blk = ctx.enter_context(tc.tile_pool(name="blk", bufs=2))
        ps_conv = ctx.enter_context(tc.tile_pool(name="ps_conv", bufs=2, space="PSUM"))
        ps_st = ctx.enter_context(tc.tile_pool(name="ps_st", bufs=2, space="PSUM"))
        ps_a = ctx.enter_context(tc.tile_pool(name="ps_a", bufs=1, space="PSUM"))
        ps_acc = ctx.enter_context(tc.tile_pool(name="ps_acc", bufs=2, space="PSUM"))
        ps_sm = ctx.enter_context(tc.tile_pool(name="ps_sm", bufs=1, space="PSUM"))

        # ---- constants ----
        ones_f = consts.tile([P, 1], F32, tag="ones_f", name="ones_f")
        nc.vector.memset(ones_f, 1.0)
        ones1_f = consts.tile([1, P], F32, tag="ones1_f", name="ones1_f")
        nc.vector.memset(ones1_f, 1.0)
        ones1_r = consts.tile([1, P], F32R, tag="ones1_r", name="ones1_r")
        nc.vector.tensor_copy(out=ones1_r, in_=ones1_f)
        eps_t = consts.tile([1, 1], F32, tag="eps", name="eps")
        nc.vector.memset(eps_t, EPS)
        ident_f = consts.tile([P, P], F32, tag="ident_f", name="ident_f")
        from concourse.masks import make_identity
        make_identity(nc, ident_f)
        ident_r = consts.tile([P, P], F32R, tag="ident_r", name="ident_r")
        nc.vector.tensor_copy(out=ident_r, in_=ident_f)
        lnke_t = consts.tile([P, 1], F32, tag="lnke", name="lnke")
        nc.vector.memset(lnke_t, float(np.log(KE)))

        def loadw(dram_ap, tag, dt):
            t = wp.tile(list(dram_ap.shape), dt, tag=tag, name=tag)
            nc.sync.dma_start(out=t, in_=dram_ap)
            return t

        win8_sb = [loadw(win8_d[i], f"win{i}", F8) for i in range(27)]
        csc_sb = [loadw(csc_d[i], f"csc{i}", F32) for i in range(3)]
        cbi_sb = [loadw(cbi_d[i], f"cbi{i}", F32) for i in range(3)]
        wkd_sb = loadw(wkd_d, "wkd", F32)
        wvd_sb = loadw(wvd_d, "wvd", F32)
        rab_sb = loadw(rab_d, "rab", F32)
        woa8_sb = [loadw(woa8_d[i], f"woa{i}", F8) for i in range(2)]
        wou8_sb = [loadw(wou8_d[i], f"wou{i}", F8) for i in range(2)]
        invlo_sb = [loadw(invlo_d[i], f"invlo{i}", F32) for i in range(2)]
        w18_sb = [loadw(w18_d[i], f"w18_{i}", F8) for i in range(4)]
        hsc_sb = [loadw(hsc_d[i], f"hsc{i}", F32) for i in range(4)]
        hbi_sb = [loadw(hbi_d[i], f"hbi{i}", F32) for i in range(4)]
        w28_sb = [loadw(w28_d[i], f"w28_{i}", F8) for i in range(4)]
        fsc_sb = [loadw(fsc_d[i], f"fsc{i}", F32) for i in range(2)]
        fbi_sb = [loadw(fbi_d[i], f"fbi{i}", F32) for i in range(2)]
        gnw_sb = [loadw(gnw_d[i], f"gnw{i}", F32) for i in range(2)]
        gnb_sb = [loadw(gnb_d[i], f"gnb{i}", F32) for i in range(2)]
        ones8_sb = loadw(ones8_d, "ones8", F8)
        ident8_sb = loadw(ident8_d, "ident8", F8)

        # ---- persistent big tiles (x/xn double-buffered across samples) --
        x_sb = [big.tile([P, 2, NBLK, BW], F32, tag=f"x_sb{i}", name=f"x_sb{i}")
                for i in range(2)]
        xn8 = [big.tile([P, 2, NPAD], F8, tag=f"xn8_{i}", name=f"xn8_{i}")
               for i in range(2)]
        q8 = big.tile([P, N], F8, tag="q8", name="q8")
        k8 = big.tile([64, N], F8, tag="k8", name="k8")
        v8 = big.tile([P, N], F8, tag="v8", name="v8")
        ru8 = big.tile([P, 2, N], F8, tag="ru8", name="ru8")
        kf8 = big.tile([P, M], F8, tag="kf8", name="kf8")
        kfa = big.tile([64, M], F32, tag="kfa", name="kfa")
        vfa = big.tile([P, M], F32R, tag="vfa", name="vfa")
        vfT8 = [big.tile([P, 2, P], F8, tag=f"vfT{j}", name=f"vfT{j}")
                for j in range(4)]

        # zero the xn8 padding once (interior is overwritten every sample),
        # and the K-padding halves of q8/kf8 (only 0:64 is ever written)
        for i in range(2):
            nc.vector.memset(xn8[i].bitcast(F32), 0.0)
        nc.vector.memset(q8[64:, :].bitcast(F32), 0.0)
        nc.vector.memset(kf8[64:, :].bitcast(F32), 0.0)

        xn8v = [t.rearrange("p ct (h w) -> p ct h w", w=WP) for t in xn8]

        ESCALE = SCALE / (KQ * KK)
        AB = [None, None]  # per-sample GN affine [A0,B0,A1,B1]

        # ---------------- stage builders (shared closures) ----------------
        def dma_stats(s, stats, ct, b):
            nc.sync.dma_start(out=x_sb[s][:, ct, b, :],
                              in_=xv_d[s, ct, :, b * BW:(b + 1) * BW])
            nc.vector.bn_stats(out=stats[ct][:, b, :], in_=x_sb[s][:, ct, b, :])

        def gn_reduce(s, stats):
            packed = small.tile([P, 4], F32, tag="packed", name="packed")
            for ct in range(2):
                mv = small.tile([P, 2], F32, tag=f"mv{ct}", name=f"mv{ct}")
                nc.vector.bn_aggr(out=mv, in_=stats[ct])
                nc.vector.tensor_copy(out=packed[:, 2 * ct:2 * ct + 1],
                                      in_=mv[:, 0:1])
                m2 = small.tile([P, 1], F32, tag=f"m2{ct}", name=f"m2{ct}")
                nc.vector.tensor_mul(out=m2, in0=mv[:, 0:1], in1=mv[:, 0:1])
                nc.vector.tensor_add(out=packed[:, 2 * ct + 1:2 * ct + 2],
                                     in0=mv[:, 1:2], in1=m2)
            gsum = ps_sm.tile([1, 4], F32, tag="sm", name="gn1", space="PSUM")
            nc.tensor.matmul(gsum, ones_f, packed, start=True, stop=True)
            gs = small.tile([1, 4], F32, tag="gs", name="gs")
            nc.vector.tensor_copy(out=gs, in_=gsum)
            sc = small.tile([1, 4], F32, tag="sc", name="sc")
            nc.vector.tensor_add(out=sc[:, 0:1], in0=gs[:, 0:1], in1=gs[:, 2:3])
            nc.scalar.mul(out=sc[:, 0:1], in_=sc[:, 0:1], mul=1.0 / DIM)
            nc.vector.tensor_add(out=sc[:, 1:2], in0=gs[:, 1:2], in1=gs[:, 3:4])
            nc.scalar.mul(out=sc[:, 1:2], in_=sc[:, 1:2], mul=1.0 / DIM)
            mu2 = small.tile([1, 1], F32, tag="mu2", name="mu2")
            nc.vector.tensor_mul(out=mu2, in0=sc[:, 0:1], in1=sc[:, 0:1])
            nc.vector.tensor_tensor(out=sc[:, 2:3], in0=sc[:, 1:2], in1=mu2,
                                    op=OP.subtract)
            # rstd = exp(-0.5*ln(var+eps))
            nc.scalar.activation(out=sc[:, 3:4], in_=sc[:, 2:3], func=AF.Ln,
                                 bias=eps_t, scale=1.0)
            rv = small.tile([1, 2], F32, tag="rv", name="rv")
            nc.vector.tensor_copy(out=rv[:, 0:1], in_=sc[:, 0:1])
            nc.scalar.activation(out=rv[:, 1:2], in_=sc[:, 3:4], func=AF.Exp,
                                 scale=-0.5)
            gbc = ps_sm.tile([P, 2], F32, tag="sm", name="gn2", space="PSUM")
            nc.tensor.matmul(gbc, ones1_f, rv, start=True, stop=True)
            A = [small.tile([P, 1], F32, tag=f"A{ct}", name=f"A{ct}")
                 for ct in range(2)]
            B = [small.tile([P, 1], F32, tag=f"B{ct}", name=f"B{ct}")
                 for ct in range(2)]
            for ct in range(2):
                nc.vector.tensor_mul(out=A[ct], in0=gnw_sb[ct], in1=gbc[:, 1:2])
                tmp = small.tile([P, 1], F32, tag=f"ab{ct}", name=f"ab{ct}")
                nc.vector.tensor_mul(out=tmp, in0=A[ct], in1=gbc[:, 0:1])
                nc.vector.tensor_tensor(out=B[ct], in0=gnb_sb[ct], in1=tmp,
                                        op=OP.subtract)
            AB[s] = (A, B)

        def xn8_write(s, ct, b):
            A, B = AB[s]
            nc.vector.tensor_scalar(
                out=xn8v[s][:, ct, 1 + RB * b:1 + RB * (b + 1), 1:W + 1],
                in0=x_sb[s][:, ct, b, :].rearrange("p (h w) -> p h w", w=W),
                scalar1=A[ct], scalar2=B[ct], op0=OP.mult, op1=OP.add)

        def p12_steps(s):
            """Emission callables: 16 dma+stats, 1 gn reduce, 16 xn writes."""
            stats = [small.tile([P, NBLK, 6], F32, tag=f"stats{ct}",
                                name=f"stats{ct}") for ct in range(2)]
            steps = []
            for ct in range(2):
                for b in range(NBLK):
                    steps.append(lambda ct=ct, b=b: dma_stats(s, stats, ct, b))
            steps.append(lambda: gn_reduce(s, stats))
            for ct in range(2):
                for b in range(NBLK):
                    steps.append(lambda ct=ct, b=b: xn8_write(s, ct, b))
            return steps

        def conv_tile(s, t, b, drains):
            y0 = RB * b
            pt = ps_conv.tile([P, BW], F32, tag="conv", name="conv",
                              space="PSUM")
            for ky in range(3):
                for kx in range(3):
                    tap = ky * 3 + kx
                    nc.tensor.matmul(
                        pt, win8_sb[tap * 3 + t],
                        xn8v[s][:, :, y0 + ky:y0 + ky + RB, kx:kx + W],
                        start=(tap == 0), stop=(tap == 8), perf_mode=DRM)
            drains(pt, b)

        def t1_drain(pt, b):
            nr = slice(b * BW, (b + 1) * BW)
            nc.vector.tensor_scalar(out=k8[:, nr], in0=pt[0:64, :],
                                    scalar1=csc_sb[1][0:64], scalar2=None,
                                    op0=OP.mult)
            nc.scalar.activation(out=ru8[64:, 1, nr], in_=pt[64:, :],
                                 func=AF.Relu, bias=cbi_sb[1][64:],
                                 scale=csc_sb[1][64:])

        def t2_drain(pt, b):
            nr = slice(b * BW, (b + 1) * BW)
            nc.vector.tensor_scalar(out=v8[:, nr], in0=pt,
                                    scalar1=csc_sb[2], scalar2=None,
                                    op0=OP.mult)

        def t0_drain(pt, b):
            nr = slice(b * BW, (b + 1) * BW)
            nc.vector.tensor_scalar(out=q8[0:64, nr], in0=pt[0:64, :],
                                    scalar1=csc_sb[0][0:64],
                                    scalar2=cbi_sb[0][0:64],
                                    op0=OP.mult, op1=OP.add)
            nc.scalar.activation(out=ru8[64:, 0, nr], in_=pt[64:, :],
                                 func=AF.Relu, bias=cbi_sb[0][64:],
                                 scale=csc_sb[0][64:])

        def phase_a_conv(s):
            for b in range(NBLK):
                conv_tile(s, 1, b, t1_drain)
            for b in range(NBLK):
                conv_tile(s, 2, b, t2_drain)

        def phase_a_post(s):
            """depthwise downsample (DVE) + vf transposes"""
            for (acc, out8, src, wsc, np_, w2_) in (
                    (kfa, kf8, k8, wkd_sb, 64, 32),
                    (vfa, None, v8, wvd_sb, P, 32)):
                sv = src.rearrange("p (h w) -> p h w", w=W)
                av = acc.rearrange("p (h w) -> p h w", w=w2_)
                for j, (dy, dx) in enumerate(((0, 0), (0, 1), (1, 0), (1, 1))):
                    sj = sv[:, dy::2, dx::2]
                    if j == 0:
                        nc.vector.tensor_scalar(out=av, in0=sj,
                                                scalar1=wsc[0:np_, 0:1],
                                                scalar2=None, op0=OP.mult)
                    else:
                        nc.vector.scalar_tensor_tensor(
                            out=av, in0=sj, scalar=wsc[0:np_, j:j + 1], in1=av,
                            op0=OP.mult, op1=OP.add)
                if out8 is not None:
                    nc.vector.tensor_copy(out=out8[0:64, :], in_=acc)
            for mt in range(8):
                ptr = ps_sm.tile([P, P], F32R, tag="sm", name="tpose",
                                 space="PSUM")
                nc.tensor.transpose(ptr, vfa[:, mt * P:(mt + 1) * P], ident_r)
                nc.vector.tensor_copy(out=vfT8[mt // 2][:, mt % 2, :], in_=ptr)

        def units_stconv(s, b_st, b_cv, est_t):
            """Emission units: score-mm+exp for block b_st zippered with the
            T0 conv taps of block b_cv (both optional)."""
            units = []
            if b_st is not None:
                nr = slice(b_st * BW, (b_st + 1) * BW)
                est8 = est_p.tile([P, 8, BW], F8, tag="est", name="est")
                est_t[b_st] = est8

                def st_unit(mt, nr=nr, est8=est8):
                    st_ps = ps_st.tile([P, BW], F32, tag="st", name="st",
                                       space="PSUM")
                    nc.tensor.matmul(st_ps, kf8[:, mt * P:(mt + 1) * P],
                                     q8[:, nr], start=True, stop=True)
                    nc.scalar.activation(out=est8[:, mt, :], in_=st_ps,
                                         func=AF.Exp, bias=lnke_t,
                                         scale=ESCALE)
                st_units = [lambda mt=mt: st_unit(mt) for mt in range(8)]
            else:
                st_units = []
            if b_cv is not None:
                y0 = RB * b_cv
                cpt_box = [None]

                def cv_unit(tap, y0=y0, b_cv=b_cv, cpt_box=cpt_box):
                    if cpt_box[0] is None:
                        cpt_box[0] = ps_conv.tile([P, BW], F32, tag="conv",
                                                  name="conv", space="PSUM")
                    ky, kx = tap // 3, tap % 3
                    nc.tensor.matmul(
                        cpt_box[0], win8_sb[tap * 3 + 0],
                        xn8v[s][:, :, y0 + ky:y0 + ky + RB, kx:kx + W],
                        start=(tap == 0), stop=(tap == 8), perf_mode=DRM)
                    if tap == 8:
                        t0_drain(cpt_box[0], b_cv)
                cv_units = [lambda tap=tap: cv_unit(tap) for tap in range(9)]
            else:
                cv_units = []
            # zip: st, cv, st, cv ... remainder appended
            n = max(len(st_units), len(cv_units))
            for i in range(n):
                if i < len(st_units):
                    units.append(st_units[i])
                if i < len(cv_units):
                    units.append(cv_units[i])
            return units

        def a_dn_mm(s, b, est_t, hold):
            """a/dn matmuls + dn row copy; the rest continues in bc_fin."""
            est8 = est_t[b]
            a_ps = ps_a.tile([P, BW], F32, tag="a", name="a", space="PSUM")
            for j in range(4):
                nc.tensor.matmul(a_ps, vfT8[j], est8[:, 2 * j:2 * j + 2, :],
                                 start=(j == 0), stop=(j == 3), perf_mode=DRM)
            dn_ps = ps_sm.tile([32, BW], F32, tag="sm", name="dn",
                               space="PSUM")
            for j in range(4):
                nc.tensor.matmul(dn_ps, ones8_sb, est8[:, 2 * j:2 * j + 2, :],
                                 start=(j == 0), stop=(j == 3), perf_mode=DRM)
            dn_r = blk.tile([1, BW], F32R, tag="dn_r", name="dn_r")
            nc.scalar.copy(out=dn_r, in_=dn_ps[0:1, :])
            hold[b] = (a_ps, dn_r)

        def bc_fin(s, b, hold, ra_t):
            a_ps, dn_r = hold[b]
            bc_ps = ps_st.tile([P, BW], F32, tag="st", name="bc", space="PSUM")
            nc.tensor.matmul(bc_ps, ones1_r, dn_r, start=True, stop=True)
            rb_sb = blk.tile([P, BW], F32, tag="rb_sb", name="rb_sb")
            nc.vector.reciprocal_approx_fast(out=rb_sb, in_=bc_ps)
            asc = blk.tile([P, BW], F32, tag="asc", name="asc")
            nc.vector.tensor_mul(out=asc, in0=a_ps, in1=rb_sb)
            ra8 = blk.tile([P, BW], F8, tag="ra8", name="ra8")
            nc.scalar.activation(out=ra8, in_=asc, func=AF.Relu, bias=rab_sb,
                                 scale=KA / KV)
            ra_t[b] = ra8

        def units_tail(s, b, ra_t):
            nr = slice(b * BW, (b + 1) * BW)
            x1f = blk.tile([P, 2, BW], F32, tag="x1f", name="x1f")
            x18 = blk.tile([P, 2, BW], F8, tag="x18", name="x18")
            h8 = blk.tile([P, 4, BW], F8, tag="h8", name="h8")

            def wout_unit(ot):
                o_ps = ps_acc.tile([P, BW], F32, tag="acc", name="o",
                                   space="PSUM")
                nc.tensor.matmul(o_ps, woa8_sb[ot], ra_t[b], start=True,
                                 stop=False)
                nc.tensor.matmul(o_ps, wou8_sb[ot][64:, :, :], ru8[64:, :, nr],
                                 start=False, stop=True, perf_mode=DRM)
                nc.vector.scalar_tensor_tensor(out=x1f[:, ot, :], in0=o_ps,
                                               scalar=invlo_sb[ot],
                                               in1=x_sb[s][:, ot, b, :],
                                               op0=OP.mult, op1=OP.add)
                nc.vector.tensor_scalar(out=x18[:, ot, :], in0=x1f[:, ot, :],
                                        scalar1=K1, scalar2=None, op0=OP.mult)

            def ffn1_unit(ot):
                h_ps = ps_acc.tile([P, BW], F32, tag="acc", name="h",
                                   space="PSUM")
                nc.tensor.matmul(h_ps, w18_sb[ot], x18, start=True, stop=True,
                                 perf_mode=DRM)
                nc.scalar.activation(out=h8[:, ot, :], in_=h_ps, func=AF.Relu,
                                     bias=hbi_sb[ot], scale=hsc_sb[ot])

            def ffn2_unit(ot):
                f_ps = ps_acc.tile([P, BW], F32, tag="acc", name="f",
                                   space="PSUM")
                nc.tensor.matmul(f_ps, w28_sb[2 * ot], h8[:, 0:2, :],
                                 start=True, stop=False, perf_mode=DRM)
                nc.tensor.matmul(f_ps, w28_sb[2 * ot + 1], h8[:, 2:4, :],
                                 start=False, stop=True, perf_mode=DRM)
                ob = blk.tile([P, BW], F32, tag=f"ob{ot}", name=f"ob{ot}")
                nc.vector._custom_dve(AFFINE_THEN_ADD, out=ob, in0=f_ps,
                                      in1=x1f[:, ot, :], s0=fsc_sb[ot],
                                      s1=fbi_sb[ot])
                nc.sync.dma_start(out=ov_d[s, ot, :, nr], in_=ob)

            return [lambda: wout_unit(0), lambda: wout_unit(1),
                    lambda: ffn1_unit(0), lambda: ffn1_unit(1),
                    lambda: ffn1_unit(2), lambda: ffn1_unit(3),
                    lambda: ffn2_unit(0), lambda: ffn2_unit(1)]

        def phase_b(s, feeder):
            """Two-deep pipelined per-block loop: block b's attention tail
            runs one iteration later, its matmuls interleaved 2:1 with the
            next block's score/conv stream so PE drain-waits are covered."""
            est_t = [None] * NBLK
            ra_t = [None] * NBLK
            hold = [None] * NBLK
            if feeder:
                cuts = [len(feeder) * i // NBLK for i in range(NBLK + 1)]
                sched = [feeder[cuts[i]:cuts[i + 1]] for i in range(NBLK)]
            else:
                sched = [[] for _ in range(NBLK)]
            for u in units_stconv(s, 0, None, est_t):
                u()
            for b in range(NBLK):
                a_dn_mm(s, b, est_t, hold)
                A = units_stconv(s, b + 1 if b + 1 < NBLK else None,
                                 b + 2 if b + 2 < NBLK else None, est_t)
                B = [lambda b=b: bc_fin(s, b, hold, ra_t)]
                if b >= 1:
                    B += units_tail(s, b - 1, ra_t)
                # interleave ~2 A units per B unit
                ia = ib = 0
                while ia < len(A) or ib < len(B):
                    for _ in range(2):
                        if ia < len(A):
                            A[ia](); ia += 1
                    if ib < len(B):
                        B[ib](); ib += 1
                for step in sched[b]:
                    step()
            for u in units_tail(s, NBLK - 1, ra_t):
                u()

        # ---------------- emission schedule ----------------
        steps0 = p12_steps(0)
        for st_ in steps0[:17]:      # dma+stats+gn for sample 0
            st_()
        # interleave sample0 xn8 writes with its T1 conv blocks
        xnw0 = steps0[17:]
        for b in range(NBLK):
            xnw0[b]()        # ct0 block b
            xnw0[8 + b]()    # ct1 block b
        phase_a_conv(0)
        conv_tile(0, 0, 0, t0_drain)
        conv_tile(0, 0, 1, t0_drain)
        phase_a_post(0)
        phase_b(0, p12_steps(1))
        phase_a_conv(1)
        conv_tile(1, 0, 0, t0_drain)
        conv_tile(1, 0, 1, t0_drain)
        phase_a_post(1)
        phase_b(1, None)

    nc.compile()
    return nc
def kernel(**inputs):
    x = np.ascontiguousarray(np.asarray(inputs["x"], dtype=np.float32))
    B = x.shape[0]
    assert B == NCORES * SPC
    w = _prep_weights(
        inputs["w_in"], inputs["s_in"], inputs["b_in"],
        inputs["w_k"], inputs["s_k"], inputs["w_v"], inputs["s_v"],
        inputs["b_v"], inputs["w_out"], inputs["s_out"], inputs["b_out"],
        inputs["w1"], inputs["s1"], inputs["b1"],
        inputs["w2"], inputs["s2"], inputs["b2"],
        inputs["gn_w"], inputs["gn_b"])
    w = {k: np.ascontiguousarray(v) for k, v in w.items()}

    if "nc" not in _CACHE:
        _CACHE["nc"] = _build(NCORES)
    nc = _CACHE["nc"]

    in_maps = []
    for c in range(NCORES):
        m = dict(w)
        m["x"] = np.ascontiguousarray(
            x[c * SPC:(c + 1) * SPC].reshape(SPC, DIM, N))
        in_maps.append(m)

    res = run_bass_kernel_spmd(nc, in_maps, list(range(NCORES)))
    _CACHE["last_result"] = res
    out = np.concatenate([r["out"] for r in res.results], axis=0)
    return out.reshape(B, DIM, H, W).astype(np.float32)


if __name__ == "__main__":
    print("building...")
    nc = _build(NCORES)
    print("built ok")
